# revision 35
# baseline (speedup 1.0000x reference)
"""BertAttention (T5-style relative-position bias) Trainium2 Bass kernel.

Strategy (8-way tensor parallel over heads, 2 heads/core), v4:
  - Host pre-transposes hidden -> hT [HID, B*S] bf16; all matmul operands bf16
    (fp32 PSUM accumulation); fp32 only for biases/normalization/output.
  - Host pre-expands the relative-bias table into a forward Toeplitz row
    tf[h, y] = exp(bias at rel = y - 2047), loaded once into an SBUF master;
    per-tile exp-bias factors are free slices of it.
  - T5 bias via softmax shift-invariance: with A_h = table[h, 15] subtracted,
    far-left tiles need no bias; far-right tiles use a pre-scaled V copy
    (exp(t31-t15), incl. the ones/denominator column); middle tiles get a DVE
    multiply of exp-scores by the master slice (reversed free dim).
  - One unified instruction pipeline: a single shared PSUM ring ('d', 2 banks)
    serves QKV accumulation, V transposes (bitcast bf16), the normalization
    broadcast, and the dense pass, so no pool-scope reuse dependency ever
    serializes phases. Attention PSUM: scores 2x[128,1024] + ctx 2x[65,512].
  - Emission interleaves batch-0 projection chunks with batch-0 qc0 attention
    k-quarters (ACT starts exping within ~10us), batch-1 projection chunks in
    later qc gaps, and batch-0's a2a/dense inside batch-1 attention.
  - Softmax denominator via ones-column in V; normalization uses a K=1 PE
    matmul to broadcast reciprocals (no gpsimd, keeps the Pool queue free for
    the two AllToAll collectives).
  - DMAs split across the SP and Activation HWDGE queues.
"""
import sys
import math
import os

sys.path.insert(0, "/opt/trn_rl_repo")

import numpy as np
import ml_dtypes

import concourse.bass as bass
import concourse.bacc as bacc
import concourse.tile as tile
import concourse.mybir as mybir
from concourse.bass_utils import run_bass_kernel_spmd
from concourse.masks import make_identity

F32 = mybir.dt.float32
BF16 = mybir.dt.bfloat16
Exp = mybir.ActivationFunctionType.Exp
ADD = mybir.AluOpType.add
MULT = mybir.AluOpType.mult

B, S, HID = 2, 2048, 1024
NH, HD = 16, 64
NB, MAXD = 32, 128
N_CORES = 8
HPC = NH // N_CORES          # heads per core = 2
T = B * S                    # 4096 flat tokens
FEAT = 3 * HPC * HD          # 384 qkv features per core
KTILES = S // 128            # 16 k tiles per batch
QCH = S // 512               # 4 q chunks of 512 per batch
TW = 4096                    # padded width of expanded bias table (0..4094 used)
MW = 3968                    # width of the in-SBUF Toeplitz master per head
HB = S // N_CORES            # 256 tokens per core per batch

# timing-diagnostic escape hatch: build without collectives (wrong results)
_NO_COLLECTIVES = bool(os.environ.get("KERNEL_NO_COLLECTIVES"))


def _bucket(rel):
    """T5 bidirectional bucket for array of relative positions (k - q)."""
    rel = np.asarray(rel, dtype=np.int64)
    nb = NB // 2
    base = np.where(rel > 0, nb, 0)
    r = np.abs(rel)
    max_exact = nb // 2
    is_small = r < max_exact
    tmp = np.log(np.maximum(r, 1).astype(np.float32) / np.float32(max_exact))
    large = tmp / np.float32(math.log(MAXD / max_exact)) * np.float32(nb - max_exact)
    large_i = max_exact + large.astype(np.int32)
    large_i = np.minimum(large_i, nb - 1)
    return (base + np.where(is_small, r, large_i)).astype(np.int32)


def _tile_classes():
    """Per (qc, kt): 'L' (bucket 15 everywhere), 'R' (bucket 31), or 'M'."""
    bm = _bucket(np.arange(-4095, 4096))

    def cls(qc, kt):
        rmin = kt * 128 - (qc * 512 + 511)
        rmax = kt * 128 + 127 - qc * 512
        sl = bm[rmin + 4095: rmax + 4096]
        if np.all(sl == 15):
            return "L"
        if np.all(sl == 31):
            return "R"
        return "M"

    return {(qc, kt): cls(qc, kt) for qc in range(QCH) for kt in range(KTILES)}


def _build_program():
    nc = bacc.Bacc("TRN2", target_bir_lowering=False, debug=False,
                   enable_asserts=True, num_devices=N_CORES)

    hT_d = nc.dram_tensor("hT", [HID, T], BF16, kind="ExternalInput")
    wq_d = nc.dram_tensor("wq", [HID, FEAT], BF16, kind="ExternalInput")
    bq_d = nc.dram_tensor("bq", [FEAT, 1], F32, kind="ExternalInput")
    wd_d = nc.dram_tensor("wd", [HID, HID], BF16, kind="ExternalInput")
    bd_d = nc.dram_tensor("bd", [HID, 1], F32, kind="ExternalInput")
    tf_d = nc.dram_tensor("tf", [HPC, TW], BF16, kind="ExternalInput")
    vs_d = nc.dram_tensor("vs", [HPC, 1], F32, kind="ExternalInput")
    out_d = nc.dram_tensor("outT", [HID, T // N_CORES], F32, kind="ExternalOutput")

    tcls = _tile_classes()
    # far tiles first (middle-tile DVE work late); qc0 of the interleaved batch
    # is chunk-ordered so each k-quarter only needs one projection chunk
    kt_order = {qc: sorted(range(KTILES), key=lambda kt: tcls[(qc, kt)] == "M")
                for qc in range(QCH)}

    with tile.TileContext(nc) as tc:
        with tc.tile_pool(name="const", bufs=1) as cst, \
             tc.tile_pool(name="big", bufs=1) as big, \
             tc.tile_pool(name="htp", bufs=16) as htp, \
             tc.tile_pool(name="expp", bufs=10) as expp, \
             tc.tile_pool(name="dns", bufs=8) as dns, \
             tc.tile_pool(name="dno", bufs=1) as dno, \
             tc.tile_pool(name="sps", bufs=2, space="PSUM") as sps, \
             tc.tile_pool(name="cps", bufs=2, space="PSUM") as cps, \
             tc.tile_pool(name="dps", bufs=2, space="PSUM") as dps, \
             tc.tile_pool(name="dram", bufs=1, space="DRAM") as dram:

            # ---------------- constants (ACT queue, small) ----------------
            ident_f = cst.tile([128, 128], F32, tag="identf")
            make_identity(nc, ident_f[:])
            identb = cst.tile([128, 128], BF16, tag="identb")
            nc.vector.tensor_copy(identb[:], ident_f[:])
            ones_b = cst.tile([128, 1], BF16, tag="ones")
            nc.gpsimd.memset(ones_b[:], 1.0)
            onesrow = cst.tile([1, 128], BF16, tag="onesrow")
            nc.gpsimd.memset(onesrow[:], 1.0)
            wq_sb = cst.tile([128, 8, FEAT], BF16, tag="wq")
            nc.scalar.dma_start(wq_sb[:], wq_d[:, :].rearrange("(j p) f -> p j f", p=128))
            bq_sb = cst.tile([128, 3, 1], F32, tag="bq")
            nc.scalar.dma_start(bq_sb[:], bq_d[:, :].rearrange("(m p) o -> p m o", p=128))
            bd_sb = cst.tile([128, 8, 1], F32, tag="bd")
            nc.scalar.dma_start(bd_sb[:], bd_d[:, :].rearrange("(e p) o -> p e o", p=128))
            vs_sb = cst.tile([1, HPC], F32, tag="vs")
            nc.scalar.dma_start(vs_sb[:], vs_d[:, :].rearrange("h o -> o h"))
            vs_pb = cst.tile([128, HPC], F32, tag="vspb")
            nc.gpsimd.partition_broadcast(vs_pb[:], vs_sb[:])

            # Toeplitz exp-bias master: master[p, h, u] = tf[h, p + u]
            master = big.tile([128, HPC, MW], BF16, tag="master")
            nc.scalar.dma_start(
                master[:], bass.AP(tf_d, 0, [[1, 128], [TW, HPC], [1, MW]]))

            # persistent per-batch tensors
            QTb = [big.tile([128, S], BF16, tag=f"QT{b}", name=f"QT{b}") for b in range(B)]
            KTb = [big.tile([128, S], BF16, tag=f"KT{b}", name=f"KT{b}") for b in range(B)]
            VTb = [big.tile([128, S], BF16, tag=f"VT{b}", name=f"VT{b}") for b in range(B)]
            Vaugb = [big.tile([128, KTILES, 130], BF16, tag=f"Vaug{b}", name=f"Vaug{b}")
                     for b in range(B)]
            Vscb = [big.tile([128, KTILES, 130], BF16, tag=f"Vsc{b}", name=f"Vsc{b}")
                    for b in range(B)]
            ctxRb = [big.tile([128, S], BF16, tag=f"ctxR{b}", name=f"ctxR{b}")
                     for b in range(B)]
            ctxTb = [big.tile([128, S], BF16, tag=f"ctxT{b}", name=f"ctxT{b}")
                     for b in range(B)]
            den_sb = [big.tile([1, HPC, S], BF16, tag=f"den{b}", name=f"den{b}")
                      for b in range(B)]
            wd_sb = big.tile([128, 8, HID], BF16, tag="wd")

            a2a_in_b = [dram.tile([HID, HB], BF16, name=f"a2ain{b}") for b in range(B)]
            a2a_out_b = [dram.tile([HID, HB], BF16, name=f"a2aout{b}") for b in range(B)]
            outT_sb = dno.tile([128, 8, 2 * HB], F32, tag="outT")

            def emit_ones(b):
                nc.vector.tensor_copy(
                    Vaugb[b][:].rearrange("p t (g c) -> p t g c", c=65)[:, :, :, 64:65],
                    ones_b[:, 0:1].to_broadcast([128, KTILES, 2, 1]))

            def proj_chunk_pieces(b, tci):
                """The chunk's work as ~1us emit-closures, for interleaving."""
                gci = b * (S // 512) + tci
                hts = []
                dests = (QTb[b], KTb[b], VTb[b])

                def p_load():
                    # b0 chunks split across both HWDGE queues (ACT idle then);
                    # b1 chunks stay off the ACT queue so they never contend
                    # with the exp stream
                    for kt in range(8):
                        ht = htp.tile([128, 512], BF16, tag="ht")
                        use_act = b == 0 and gci != 0 and kt % 2 == 1
                        eng = nc.scalar if use_act else nc.sync
                        eng.dma_start(ht[:], hT_d[128 * kt:128 * (kt + 1),
                                                  512 * gci:512 * (gci + 1)])
                        hts.append(ht)

                def p_m(m):
                    def go():
                        ps = dps.tile([128, 512], F32, tag="d",
                                      name=f"qkv{b}_{tci}_{m}")
                        for kt in range(8):
                            nc.tensor.matmul(ps[:],
                                             wq_sb[:, kt, m * 128:(m + 1) * 128],
                                             hts[kt][:], start=(kt == 0),
                                             stop=(kt == 7))
                        nc.vector.tensor_tensor(
                            dests[m][:, 512 * tci:512 * (tci + 1)], ps[:],
                            bq_sb[:, m, 0:1].to_broadcast([128, 512]), ADD)
                    return go

                def p_tr(t):
                    def go():
                        vslot = Vaugb[b][:, t, :].rearrange("p (g c) -> p g c", c=65)
                        tp = dps.tile([128, 512], F32, tag="d", name=f"tr{b}_{t}")
                        tpb = tp[:].bitcast(BF16)[:, 0:128]
                        nc.tensor.transpose(tpb, VTb[b][:, 128 * t:128 * (t + 1)],
                                            identb[:])
                        nc.vector.tensor_copy(
                            vslot[:, :, 0:64],
                            tpb.rearrange("p (g c) -> p g c", c=64))
                    return go

                def p_vsc():
                    for h in range(HPC):
                        nc.vector.tensor_tensor(
                            Vscb[b][:].rearrange("p t (g c) -> p t g c", c=65)
                            [:, 4 * tci:4 * (tci + 1), h, :],
                            Vaugb[b][:].rearrange("p t (g c) -> p t g c", c=65)
                            [:, 4 * tci:4 * (tci + 1), h, :],
                            vs_pb[:, h:h + 1].to_broadcast([128, 4, 65]), MULT)

                pieces = [p_load, p_m(0), p_m(1), p_m(2)]
                pieces += [p_tr(t) for t in range(4 * tci, 4 * (tci + 1))]
                pieces.append(p_vsc)
                return pieces

            def emit_proj_chunk(b, tci):
                for p in proj_chunk_pieces(b, tci):
                    p()

            def emit_qk_exp(b, qc, kt):
                q0 = qc * 512
                k0 = kt * 128
                s_ps = sps.tile([128, 1024], F32, tag="S")
                for h in range(HPC):
                    nc.tensor.matmul(s_ps[:, 512 * h:512 * (h + 1)],
                                     KTb[b][64 * h:64 * h + 64, k0:k0 + 128],
                                     QTb[b][64 * h:64 * h + 64, q0:q0 + 512],
                                     start=True, stop=True)
                es = expp.tile([128, 1024], BF16, tag="es")
                nc.scalar.activation(es[:], s_ps[:], Exp)
                if tcls[(qc, kt)] == "M":
                    # exp-bias Toeplitz factor; master read with a reversed
                    # free dim so rel = p + w0 - j - 2047 = k - q
                    w0 = 2047 + 128 * kt - q0
                    mview = bass.AP(
                        master.tensor, master.offset + w0,
                        [[HPC * MW, 128], [MW, HPC], [-1, 512]])
                    es2 = expp.tile([128, 1024], BF16, tag="es")
                    nc.vector.tensor_tensor(
                        es2[:].rearrange("p (h q) -> p h q", q=512),
                        es[:].rearrange("p (h q) -> p h q", q=512),
                        mview, MULT)
                    es = es2
                return es

            def emit_pv(b, qc, kt, es, ctx_ps, i):
                vsrc = Vscb[b] if tcls[(qc, kt)] == "R" else Vaugb[b]
                for h in range(HPC):
                    nc.tensor.matmul(ctx_ps[h][:],
                                     vsrc[:, kt, 65 * h:65 * h + 65],
                                     es[:, 512 * h:512 * (h + 1)],
                                     start=(i == 0), stop=(i == KTILES - 1))

            def emit_att_kts(b, qc, kts, ctx_ps, i0, inserts=None):
                for i, kt in enumerate(kts, start=i0):
                    if inserts:
                        for fn in inserts.pop(i - i0, []):
                            fn()
                    es = emit_qk_exp(b, qc, kt)
                    emit_pv(b, qc, kt, es, ctx_ps, i)

            def emit_att_finish(b, qc, ctx_ps):
                q0 = qc * 512
                for h in range(HPC):
                    with nc.allow_low_precision(reason="bf16 softmax denom"):
                        nc.vector.reciprocal(den_sb[b][0:1, h, q0:q0 + 512],
                                             ctx_ps[h][64:65, :])
                    nc.vector.tensor_copy(
                        ctxRb[b][64 * h:64 * h + 64, q0:q0 + 512],
                        ctx_ps[h][0:64, :])

            def emit_norm_qc(b, qc):
                # K=1 PE broadcast + DVE multiply + a2a input slice; deferred
                # into the next qc's stream so it never gates the next QK
                q0 = qc * 512
                for h in range(HPC):
                    bc = dps.tile([128, 512], F32, tag="d", name=f"bc{b}_{h}_{qc}")
                    nc.tensor.matmul(bc[:], onesrow[:],
                                     den_sb[b][0:1, h, q0:q0 + 512],
                                     start=True, stop=True)
                    nc.vector.tensor_tensor(
                        ctxTb[b][64 * h:64 * h + 64, q0:q0 + 512],
                        ctxRb[b][64 * h:64 * h + 64, q0:q0 + 512],
                        bc[64 * h:64 * h + 64, :], MULT)
                nc.sync.dma_start(
                    a2a_in_b[b][:].rearrange("(j p) t -> p j t", p=128)
                    [:, 2 * qc:2 * qc + 2, :],
                    ctxTb[b][:, q0:q0 + 512].rearrange("p (j t) -> p j t", t=HB))

            def new_ctx(b, qc):
                return [cps.tile([65, 512], F32, tag="ctx", name=f"ctx{h}_{b}_{qc}")
                        for h in range(HPC)]

            def emit_att_qc(b, qc, pieces=(), kts=None, span=None):
                kts = kts or kt_order[qc]
                ctx_ps = new_ctx(b, qc)
                inserts = {}
                if pieces:
                    sp = span if span is not None else len(kts) - 1
                    for i, p in enumerate(pieces):
                        inserts.setdefault(1 + (i * sp) // len(pieces), []).append(p)
                emit_att_kts(b, qc, kts, ctx_ps, 0, inserts)
                for fns in inserts.values():
                    for fn in fns:
                        fn()
                emit_att_finish(b, qc, ctx_ps)

            def emit_collective(b):
                if not _NO_COLLECTIVES:
                    nc.gpsimd.collective_compute(
                        "AllToAll", mybir.AluOpType.bypass,
                        replica_groups=[list(range(N_CORES))],
                        ins=[a2a_in_b[b][:].opt()], outs=[a2a_out_b[b][:].opt()])

            def emit_dense(b):
                # all 8 j-tiles loaded once; 4 passes of 2 e-blocks, each
                # accumulation group owning a full PSUM bank
                cfs = []
                for j in range(8):
                    cf = dns.tile([128, HB], BF16, tag="cf", name=f"cf{b}_{j}")
                    nc.sync.dma_start(cf[:], a2a_out_b[b][128 * j:128 * (j + 1), :])
                    cfs.append(cf)
                for half in range(4):
                    psb = [dps.tile([128, 512], F32, tag="d",
                                    name=f"dn{b}_{half}_{g}") for g in range(2)]
                    for j in range(8):
                        for g in range(2):
                            eg = 2 * half + g
                            nc.tensor.matmul(psb[g][:, 0:HB],
                                             wd_sb[:, j, 128 * eg:128 * (eg + 1)],
                                             cfs[j][:], start=(j == 0), stop=(j == 7))
                    for g in range(2):
                        eg = 2 * half + g
                        nc.vector.tensor_tensor(
                            outT_sb[:, eg, b * HB:(b + 1) * HB], psb[g][:, 0:HB],
                            bd_sb[:, eg, 0:1].to_broadcast([128, HB]), ADD)
                        nc.sync.dma_start(
                            out_d[:, :].rearrange("(e p) t -> p e t", p=128)
                            [:, eg, b * HB:(b + 1) * HB],
                            outT_sb[:, eg, b * HB:(b + 1) * HB])

            # ---------------- unified pipeline ----------------
            emit_ones(0)
            emit_ones(1)
            # b0 projection: chunk 0 upfront, chunk q+1's pieces spread inside
            # qc0's quarter q (quarter q only needs chunk q)
            ctx0 = new_ctx(0, 0)
            c0 = proj_chunk_pieces(0, 0)
            for p in c0[:3]:                      # load + Q + K m-groups
                p()
            es_head = [emit_qk_exp(0, 0, kt) for kt in range(4)]
            for p in c0[3:]:                      # V m-group, transposes, vsc
                p()
            for kt in range(4):
                emit_pv(0, 0, kt, es_head[kt], ctx0, kt)
            q_ins = [proj_chunk_pieces(0, 2), proj_chunk_pieces(0, 3), []]
            for p in proj_chunk_pieces(0, 1):
                p()
            for q in range(1, 4):
                kts = [4 * q + 0, 4 * q + 1, 4 * q + 2, 4 * q + 3]
                ins = {i: [p] for i, p in enumerate(q_ins[q - 1][:4])}
                emit_att_kts(0, 0, kts, ctx0, 4 * q, ins)
                for p in q_ins[q - 1][4:]:
                    p()
            emit_att_finish(0, 0, ctx0)
            # remaining b0 qc blocks with b1 projection pieces and the previous
            # qc's deferred normalization in the stream
            emit_att_qc(0, 1, proj_chunk_pieces(1, 0) + [lambda: emit_norm_qc(0, 0)])
            emit_att_qc(0, 2, proj_chunk_pieces(1, 1) + [lambda: emit_norm_qc(0, 1)])
            emit_att_qc(0, 3, proj_chunk_pieces(1, 2) + [lambda: emit_norm_qc(0, 2)])
            # b1/qc0: chunk-3-dependent k tiles last; chunk(1,3) pieces inside
            kts_b1q0 = [kt for kt in kt_order[0] if kt < 12] + \
                       [kt for kt in kt_order[0] if kt >= 12]
            pieces = proj_chunk_pieces(1, 3)
            pieces.append(lambda: emit_norm_qc(0, 3))
            pieces.append(lambda: emit_collective(0))
            pieces.append(lambda: nc.scalar.dma_start(
                wd_sb[:], wd_d[:, :].rearrange("(j p) e -> p j e", p=128)))
            emit_att_qc(1, 0, pieces, kts=kts_b1q0, span=9)
            emit_att_qc(1, 1, [lambda: emit_norm_qc(1, 0)])
            emit_att_qc(1, 2, [lambda: emit_norm_qc(1, 1)])
            emit_att_qc(1, 3, [lambda: emit_norm_qc(1, 2)])
            emit_norm_qc(1, 3)
            emit_collective(1)
            emit_dense(0)
            emit_dense(1)

    nc.compile()
    return nc


_NC_CACHE = None
_TF_CACHE = None


def _bucket_fwd_cached():
    global _TF_CACHE
    if _TF_CACHE is None:
        _TF_CACHE = _bucket(np.arange(TW - 1) - 2047)   # fwd[y] = bucket(y-2047)
    return _TF_CACHE


def _get_program():
    global _NC_CACHE
    if _NC_CACHE is None:
        _NC_CACHE = _build_program()
    return _NC_CACHE


def build_inmaps(inputs):
    hidden_states = np.asarray(inputs["hidden_states"], dtype=np.float32)
    w_qkv = np.asarray(inputs["w_qkv"], dtype=np.float32)
    b_qkv = np.asarray(inputs["b_qkv"], dtype=np.float32)
    w_dense = np.asarray(inputs["w_dense"], dtype=np.float32)
    b_dense = np.asarray(inputs["b_dense"], dtype=np.float32)
    rel_attn_table = np.asarray(inputs["rel_attn_table"], dtype=np.float32)

    hT = np.ascontiguousarray(
        hidden_states.reshape(T, HID).T).astype(ml_dtypes.bfloat16)   # [HID, T]
    wd_b = np.ascontiguousarray(w_dense).astype(ml_dtypes.bfloat16)
    bmf = _bucket_fwd_cached()

    scale = np.float32(1.0 / math.sqrt(HD))
    in_maps = []
    for c in range(N_CORES):
        ha, hb = HPC * c, HPC * c + 1
        cols = []
        bias = []
        for blk, sc in ((0, scale), (1, np.float32(1.0)), (2, np.float32(1.0))):
            for h in (ha, hb):
                sl = slice(blk * HID + h * HD, blk * HID + (h + 1) * HD)
                cols.append(w_qkv[:, sl] * sc)
                bias.append(b_qkv[sl] * sc)
        wq_c = np.ascontiguousarray(
            np.concatenate(cols, axis=1)).astype(ml_dtypes.bfloat16)     # [HID, 384]
        bq_c = np.concatenate(bias).reshape(FEAT, 1).astype(np.float32)
        # shifted exp-bias: exp(t - t[:,15]); forward-expanded Toeplitz row
        tloc = rel_attn_table[ha:hb + 1]                                 # [2, NB]
        et = np.exp(tloc - tloc[:, 15:16]).astype(np.float32)
        tf = np.zeros((HPC, TW), dtype=np.float32)
        tf[:, :TW - 1] = et[:, bmf]
        in_maps.append({
            "hT": hT,
            "wq": wq_c,
            "bq": bq_c,
            "wd": wd_b,
            "bd": b_dense.reshape(HID, 1),
            "tf": tf.astype(ml_dtypes.bfloat16),
            "vs": et[:, 31:32].copy(),
        })
    return in_maps


def kernel(hidden_states, w_qkv, b_qkv, w_dense, b_dense, rel_attn_table):
    in_maps = build_inmaps(dict(
        hidden_states=hidden_states, w_qkv=w_qkv, b_qkv=b_qkv,
        w_dense=w_dense, b_dense=b_dense, rel_attn_table=rel_attn_table))

    nc = _get_program()
    res = run_bass_kernel_spmd(nc, in_maps, core_ids=list(range(N_CORES)))
    full = np.empty((HID, T), dtype=np.float32)
    for c in range(N_CORES):
        o = res.results[c]["outT"]            # [HID, 2*HB]: [b0 block c | b1 block c]
        full[:, c * HB:(c + 1) * HB] = o[:, :HB]
        full[:, S + c * HB:S + (c + 1) * HB] = o[:, HB:]
    return np.ascontiguousarray(full.T).reshape(B, S, HID)
